# revision 1
# baseline (speedup 1.0000x reference)
"""Trainium2 Bass kernel for 2-layer GAT (nn_GAT_86535001080291).

Strategy (dst-sharded graph parallelism over 8 NeuronCores):
  - Core r owns destination nodes [r*12500, (r+1)*12500).
  - Per-node "table" rows (256B, bf16) hold per-node quantities:
      layer1: [h0(32) | 1 | h1(32) | 1 | s0 s1 d0 d1]   (h = x@W1 heads,
               s/d = per-node attention logit halves)
      layer2: [h2(32) | 1 | s2 | d2]
    Table1 is built replicated on every core from x; table2 is built for the
    own shard and AllGather'd.
  - Edges (with self-loops) are grouped per core into supertiles of 128 dst
    nodes; per (supertile, src-bucket) runs are padded to 128-edge chunks
    (src buckets of 25088 table rows keep dma_gather's int16 indices in
    range). Per-edge source rows are fetched with nc.gpsimd.dma_gather.
  - Host-precomputed structure matrices (graph-only): M0 (one-hot
    [128 edge, 128 dst] fp8) and its transpose M0T stream from DRAM.
      t = s_src + d_dst        : d-expansion via PE matmul M0T.T @ d  (+ s)
      ex = exp(leakyrelu(t))   : DVE + ACT (no segment max needed; logits
                                 are bounded by construction, exp is safe)
      messages *= ex           : DVE broadcast multiply (in-place, incl. the
                                 ones-column -> denominator)
      agg[dst]  = M0.T @ msgs  : PE matmul accumulating in PSUM; the
                                 ones-column yields the softmax denominator
      out[dst]  = agg / denom  (+ bias, gelu between layers)
All host-side preprocessing depends only on edge_index (graph structure).
"""
import math
from dataclasses import dataclass, field

import numpy as np
import ml_dtypes

import concourse.bacc as bacc
import concourse.mybir as mybir
from concourse.tile import TileContext
from concourse.masks import make_identity
from concourse.tile_rust import add_dep_helper
from concourse import library_config

F32 = mybir.dt.float32
BF16 = mybir.dt.bfloat16
FP8 = mybir.dt.float8e4
I16 = mybir.dt.int16
P = 128


@dataclass
class Cfg:
    N: int = 100000
    E: int = 1600000
    IN: int = 64
    HID: int = 32
    HEADS: int = 2
    OUT: int = 32
    neg: float = 0.2
    CORES: int = 8
    GST: int = 4            # supertiles per group
    sim_gelu: bool = False  # tanh-approx gelu (CoreSim lacks Gelu LUT)

    @property
    def SHARD(self):
        return self.N // self.CORES

    @property
    def ST(self):
        return math.ceil(self.SHARD / P)

    @property
    def SHARD_PAD(self):
        return self.ST * P

    @property
    def NT(self):
        return self.CORES * self.SHARD_PAD

    @property
    def BROWS(self):
        # src bucket size in table rows (int16 gather index limit)
        nb = self.NBUCK
        return (self.NT + nb - 1) // nb

    @property
    def NBUCK(self):
        return max(1, math.ceil(self.NT / 25088))


@dataclass
class GroupSched:
    sts: list            # absolute supertile ids
    nch: int = 0         # chunks in group
    c16: int = 0         # idx columns (slots/16)
    calls: list = field(default_factory=list)   # per bucket: (off16, nidx, ch0)
    chunks: list = field(default_factory=list)  # (ci, sti, st_abs, c_st, first, last)
    struns: dict = field(default_factory=dict)  # sti -> list of (b, ch0, c0, B)
    bst: dict = field(default_factory=dict)     # sti -> total chunks for st


def build_schedule(cfg, B_sb):
    """Shared (core-independent) static schedule from the padded chunk counts."""
    groups = []
    st = 0
    while st < cfg.ST:
        sts = list(range(st, min(st + cfg.GST, cfg.ST)))
        g = GroupSched(sts=sts)
        ci = 0
        cst = {i: 0 for i in range(len(sts))}
        tot = {i: int(sum(B_sb[s][b] for b in range(cfg.NBUCK))) for i, s in enumerate(sts)}
        # slot layout: bucket-major (gather calls need contiguous runs)
        for b in range(cfg.NBUCK):
            off16 = ci * 8  # 128/16 per chunk
            ch0b = ci
            for i, s in enumerate(sts):
                B = int(B_sb[s][b])
                g.struns.setdefault(i, []).append((b, ci, cst[i], B))
                ci += B
                cst[i] += B
            nidx = (ci - ch0b) * P
            g.calls.append((off16, nidx, ch0b))
        # chunk processing order: st-major, so each supertile's PSUM
        # accumulation group opens and closes without interleaving
        for i, s in enumerate(sts):
            seen = 0
            for (b, ch0, c0, B) in g.struns[i]:
                for c in range(B):
                    first = seen == 0
                    seen += 1
                    last = seen == tot[i]
                    g.chunks.append((ch0 + c, i, s, c0 + c, first, last))
        g.nch = ci
        g.c16 = ci * 8
        g.bst = tot
        groups.append(g)
        st += cfg.GST
    return groups


def preprocess(edge_index, cfg):
    """Pure graph preprocessing: per-core gather indices + M0/M0T structure."""
    src = edge_index[0].astype(np.int64)
    dst = edge_index[1].astype(np.int64)
    loops = np.arange(cfg.N, dtype=np.int64)
    src = np.concatenate([src, loops])
    dst = np.concatenate([dst, loops])

    SH, SP, ST, NB, BR = cfg.SHARD, cfg.SHARD_PAD, cfg.ST, cfg.NBUCK, cfg.BROWS

    per_core = []
    cnt = np.zeros((cfg.CORES, ST, NB), dtype=np.int64)
    for r in range(cfg.CORES):
        m = (dst >= r * SH) & (dst < (r + 1) * SH)
        s_r = src[m]
        d_r = dst[m] - r * SH
        srow = (s_r // SH) * SP + (s_r % SH)
        b_r = srow // BR
        st_r = d_r // P
        per_core.append((srow, d_r, b_r, st_r))
        np.add.at(cnt[r], (st_r, b_r), 1)

    B_sb = np.ceil(cnt.max(axis=0) / P).astype(np.int64)  # [ST, NB]
    groups = build_schedule(cfg, B_sb)
    NG = len(groups)
    CHmax = max(g.nch for g in groups)
    C16max = max(g.c16 for g in groups)
    Bmax = int(max(max(g.bst.values()) for g in groups))

    # group-relative chunk offset of each (st, b) cell
    cell_ch0 = {}
    for gi, g in enumerate(groups):
        for i, runs in g.struns.items():
            for (b, ch0, c0, B) in runs:
                cell_ch0[(g.sts[i], b)] = (gi, ch0)

    arrays = []
    for r in range(cfg.CORES):
        srow, d_r, b_r, st_r = per_core[r]
        gi_r = st_r // cfg.GST
        # sort edges by (group, bucket, st) then stable
        order = np.lexsort((st_r, b_r, gi_r))
        srow, d_r, b_r, st_r = srow[order], d_r[order], b_r[order], st_r[order]
        gi_r = gi_r[order]

        # rank within each (st, b) cell
        cell_key = st_r * NB + b_r
        change = np.empty(len(cell_key), dtype=bool)
        change[0] = True
        change[1:] = cell_key[1:] != cell_key[:-1]
        starts = np.flatnonzero(change)
        rank = np.arange(len(cell_key)) - np.repeat(starts, np.diff(np.append(starts, len(cell_key))))

        idx16 = np.zeros((NG, 16, C16max), dtype=np.int16)
        m0 = np.zeros((NG, P, CHmax * P), dtype=np.uint8)
        m0t = np.zeros((NG, P, CHmax * P), dtype=np.uint8)
        ONE = np.float32(1.0).astype(ml_dtypes.float8_e4m3).view(np.uint8)

        gi_e = np.array([cell_ch0[(int(s), int(b))][0] for s, b in zip(st_r, b_r)]) \
            if len(st_r) else np.zeros(0, np.int64)
        ch0_e = np.array([cell_ch0[(int(s), int(b))][1] for s, b in zip(st_r, b_r)]) \
            if len(st_r) else np.zeros(0, np.int64)
        # the loop above is slow in python; vectorize via lookup tables
        gi_tab = np.zeros((ST, NB), np.int64)
        ch0_tab = np.zeros((ST, NB), np.int64)
        for (s, b), (gg, cc) in cell_ch0.items():
            gi_tab[s, b] = gg
            ch0_tab[s, b] = cc
        gi_e = gi_tab[st_r, b_r]
        ch0_e = ch0_tab[st_r, b_r]

        slot = ch0_e * P + rank                # group-relative slot
        chunk = slot // P
        pp = slot % P
        dloc = d_r % P

        idx16[gi_e, slot % 16, slot // 16] = (srow - b_r * BR).astype(np.int16)
        m0[gi_e, pp, chunk * P + dloc] = ONE
        m0t[gi_e, dloc, chunk * P + pp] = ONE

        idx128 = np.tile(idx16, (1, 8, 1))
        # per-core one-hot for own-shard d selection
        onehot = np.zeros((1, cfg.CORES), dtype=np.float32)
        onehot[0, r] = 1.0
        arrays.append(dict(
            idx=idx128,
            m0=m0.view(ml_dtypes.float8_e4m3),
            m0t=m0t.view(ml_dtypes.float8_e4m3),
            onehot=onehot,
        ))

    sched = dict(groups=groups, NG=NG, CHmax=CHmax, C16max=C16max, Bmax=Bmax)
    return sched, arrays


def build_nc(cfg, sched):
    nc = bacc.Bacc("TRN2", target_bir_lowering=False)
    NG, CHmax, C16max, Bmax = sched["NG"], sched["CHmax"], sched["C16max"], sched["Bmax"]
    groups = sched["groups"]
    ST, NT, SP, NB, BR = cfg.ST, cfg.NT, cfg.SHARD_PAD, cfg.NBUCK, cfg.BROWS
    H = cfg.HEADS

    # ---- external I/O ----
    x_ext = nc.dram_tensor("x", [cfg.N, cfg.IN], F32, kind="ExternalInput")
    W1_ext = nc.dram_tensor("W1", [cfg.IN, H * cfg.HID], F32, kind="ExternalInput")
    as1_ext = nc.dram_tensor("a_src1", [H, cfg.HID], F32, kind="ExternalInput")
    ad1_ext = nc.dram_tensor("a_dst1", [H, cfg.HID], F32, kind="ExternalInput")
    b1_ext = nc.dram_tensor("b1", [1, H * cfg.HID], F32, kind="ExternalInput")
    W2_ext = nc.dram_tensor("W2", [H * cfg.HID, cfg.OUT], F32, kind="ExternalInput")
    as2_ext = nc.dram_tensor("a_src2", [1, cfg.OUT], F32, kind="ExternalInput")
    ad2_ext = nc.dram_tensor("a_dst2", [1, cfg.OUT], F32, kind="ExternalInput")
    b2_ext = nc.dram_tensor("b2", [1, cfg.OUT], F32, kind="ExternalInput")
    idx_ext = nc.dram_tensor("idx", [NG, P, C16max], I16, kind="ExternalInput")
    m0_ext = nc.dram_tensor("m0", [NG, P, CHmax * P], FP8, kind="ExternalInput")
    m0t_ext = nc.dram_tensor("m0t", [NG, P, CHmax * P], FP8, kind="ExternalInput")
    oh_ext = nc.dram_tensor("onehot", [1, cfg.CORES], F32, kind="ExternalInput")
    out_ext = nc.dram_tensor("out", [cfg.SHARD, cfg.OUT], F32, kind="ExternalOutput")

    with TileContext(nc) as tc:
        with (
            tc.tile_pool(name="dram", bufs=1, space="DRAM") as dpool,
            tc.tile_pool(name="const", bufs=1) as cpool,
            tc.tile_pool(name="work", bufs=3) as wpool,
            tc.tile_pool(name="gath", bufs=2) as gpool,
        ):
            nc.gpsimd.load_library(library_config.mlp)

            xbf = dpool.tile([NT, P], BF16)
            table1 = dpool.tile([NT, P], BF16)
            d_all = dpool.tile([P, cfg.CORES * ST * H], BF16)
            t2_shard = dpool.tile([SP, P], BF16)
            t2_full = dpool.tile([NT, P], BF16, addr_space="Shared")

            ident = cpool.tile([P, P], F32)
            make_identity(nc, ident[:])
            _pp0cm = tc.tile_pool(name="psum0", bufs=2, space="PSUM")
            ppool0 = _pp0cm.__enter__()

            # ---------- weight prep ----------
            w1_t = cpool.tile([cfg.IN, H * cfg.HID], F32)
            nc.sync.dma_start(out=w1_t[:], in_=W1_ext[:, :])
            w2_t = cpool.tile([H * cfg.HID, cfg.OUT], F32)
            nc.sync.dma_start(out=w2_t[:], in_=W2_ext[:, :])
            # a vectors as [HID, 1] columns
            av = cpool.tile([cfg.HID, 2 * H + 2], F32)
            for h in range(H):
                nc.sync.dma_start(out=av[:, h:h + 1], in_=as1_ext[h:h + 1, :])
                nc.sync.dma_start(out=av[:, H + h:H + h + 1], in_=ad1_ext[h:h + 1, :])
            nc.sync.dma_start(out=av[:, 2 * H:2 * H + 1], in_=as2_ext[0:1, :])
            nc.sync.dma_start(out=av[:, 2 * H + 1:2 * H + 2], in_=ad2_ext[0:1, :])

            # per-head W1 transposes (base partition 0)
            w1Th = cpool.tile([cfg.HID, H, cfg.IN], F32)
            for h in range(H):
                w1Th_p = ppool0.tile([cfg.HID, cfg.IN], F32, space="PSUM", tag="prep")
                nc.tensor.transpose(out=w1Th_p[:],
                                    in_=w1_t[:, h * cfg.HID:(h + 1) * cfg.HID],
                                    identity=ident[0:cfg.IN, 0:cfg.IN])
                nc.vector.tensor_copy(out=w1Th[:, h, :], in_=w1Th_p[:])
            w2T_p = ppool0.tile([cfg.OUT, H * cfg.HID], F32, space="PSUM", tag="prep")
            nc.tensor.transpose(out=w2T_p[:], in_=w2_t[:, :],
                                identity=ident[0:H * cfg.HID, 0:H * cfg.HID])
            w2T = cpool.tile([cfg.OUT, H * cfg.HID], F32)
            nc.vector.tensor_copy(out=w2T[:], in_=w2T_p[:])

            # logit weight vectors: wv1[:, 0:2H] = per-head [src..., dst...]
            wv_p = ppool0.tile([cfg.IN, 2 * H + 2], F32, space="PSUM", tag="prep2")
            for h in range(H):
                nc.tensor.matmul(out=wv_p[:, h:h + 1],
                                 lhsT=w1Th[:, h, :],
                                 rhs=av[0:cfg.HID, h:h + 1], start=True, stop=True)
                nc.tensor.matmul(out=wv_p[:, H + h:H + h + 1],
                                 lhsT=w1Th[:, h, :],
                                 rhs=av[0:cfg.HID, H + h:H + h + 1], start=True, stop=True)
            # layer2 vectors: W2 @ a_src2 : contraction over OUT
            nc.tensor.matmul(out=wv_p[0:H * cfg.HID, 2 * H:2 * H + 1], lhsT=w2T[:, :],
                             rhs=av[0:cfg.OUT, 2 * H:2 * H + 1], start=True, stop=True)
            nc.tensor.matmul(out=wv_p[0:H * cfg.HID, 2 * H + 1:2 * H + 2], lhsT=w2T[:, :],
                             rhs=av[0:cfg.OUT, 2 * H + 1:2 * H + 2], start=True, stop=True)

            # W1ext bf16 [IN, 70]: [W1h0 | 0 | W1h1 | 0 | s0 s1 d0 d1]
            NC1 = 2 * (cfg.HID + 1) + 2 * H
            w1e = cpool.tile([cfg.IN, NC1], BF16)
            for h in range(H):
                nc.vector.tensor_copy(out=w1e[:, h * (cfg.HID + 1):h * (cfg.HID + 1) + cfg.HID],
                                      in_=w1_t[:, h * cfg.HID:(h + 1) * cfg.HID])
                nc.vector.memset(w1e[:, h * (cfg.HID + 1) + cfg.HID:(h + 1) * (cfg.HID + 1)], 0.0)
            nc.vector.tensor_copy(out=w1e[:, 2 * (cfg.HID + 1):2 * (cfg.HID + 1) + H],
                                  in_=wv_p[:, 0:H])
            nc.vector.tensor_copy(out=w1e[:, 2 * (cfg.HID + 1) + H:NC1],
                                  in_=wv_p[:, H:2 * H])
            # W2ext f32 [64, 34]: [W2 | s2vec | d2vec]
            NC2 = cfg.OUT + 2
            w2e = cpool.tile([H * cfg.HID, NC2], F32)
            nc.vector.tensor_copy(out=w2e[:, 0:cfg.OUT], in_=w2_t[:, :])
            nc.vector.tensor_copy(out=w2e[:, cfg.OUT:NC2],
                                  in_=wv_p[0:H * cfg.HID, 2 * H:2 * H + 2])

            # biases broadcast to all partitions
            b1_bc = cpool.tile([P, H, cfg.HID], F32)
            b1_row = cpool.tile([1, H * cfg.HID], F32)
            nc.sync.dma_start(out=b1_row[:], in_=b1_ext[:, :])
            nc.gpsimd.partition_broadcast(
                out_ap=b1_bc[:].rearrange("p h d -> p (h d)"), in_ap=b1_row[:])
            b2_bc = cpool.tile([P, cfg.OUT], F32)
            b2_row = cpool.tile([1, cfg.OUT], F32)
            nc.sync.dma_start(out=b2_row[:], in_=b2_ext[:, :])
            nc.gpsimd.partition_broadcast(out_ap=b2_bc[:], in_ap=b2_row[:])

            oh_row = cpool.tile([1, cfg.CORES], F32)
            nc.sync.dma_start(out=oh_row[:], in_=oh_ext[:, :])
            oh_bc = cpool.tile([P, cfg.CORES], F32)
            nc.gpsimd.partition_broadcast(out_ap=oh_bc[:], in_ap=oh_row[:])

            # ---------- phase 0a: cast x to padded bf16 rows (SBUF bounce) ----------
            for r in range(cfg.CORES):
                for st in range(ST):
                    rows = min(P, cfg.SHARD - st * P)
                    xf = wpool.tile([P, cfg.IN], F32, tag="xf")
                    nc.sync.dma_start(
                        out=xf[0:rows, :],
                        in_=x_ext[r * cfg.SHARD + st * P:
                                  r * cfg.SHARD + st * P + rows, :])
                    xb = wpool.tile([P, P], BF16, tag="xb")
                    if rows < P:
                        # zero the pad-node tail (32-aligned partition base)
                        base = (rows // 32) * 32
                        nc.vector.memset(xb[base:P, 0:cfg.IN], 0.0)
                    nc.vector.tensor_copy(out=xb[0:rows, 0:cfg.IN], in_=xf[0:rows, :])
                    nc.sync.dma_start(
                        out=xbf[r * SP + st * P:r * SP + st * P + P, 0:cfg.IN],
                        in_=xb[0:P, 0:cfg.IN])

            # ---------- phase 0b: table1 build (replicated over all regions) ----------
            t1_writes = []
            SD1 = 2 * (cfg.HID + 1)  # offset of s-cols in table1
            for r in range(cfg.CORES):
                for st in range(ST):
                    rows = cfg.SHARD - st * P if st == ST - 1 else P
                    rows = min(P, rows)
                    r0 = r * SP + st * P
                    xT = wpool.tile([P, P], BF16, tag="xT")
                    nc.sync.dma_start(out=xT[:], in_=xbf[r0:r0 + P, :], transpose=True)
                    hp = ppool0.tile([P, NC1], F32, space="PSUM", tag="hp")
                    nc.tensor.matmul(out=hp[:, :], lhsT=xT[0:cfg.IN, :],
                                     rhs=w1e[:, :], start=True, stop=True)
                    pack = wpool.tile([P, P], BF16, tag="pack")
                    nc.vector.tensor_copy(out=pack[:, 0:NC1], in_=hp[:, :])
                    ones_view = pack[:, 0:SD1].rearrange(
                        "p (h d) -> p h d", h=H)[:, :, cfg.HID:cfg.HID + 1]
                    nc.vector.memset(ones_view, 1.0)
                    if cfg.sim_gelu:  # sim-only: appease uninit-read checker
                        nc.vector.memset(pack[:, NC1:P], 0.0)
                    t1_writes.append(nc.sync.dma_start(
                        out=table1[r0:r0 + rows, :], in_=pack[0:rows, :]).ins)
                    # compact d columns for all regions (for own-shard select)
                    dpk = wpool.tile([P, H], BF16, tag="dpk")
                    nc.vector.tensor_copy(out=dpk[:], in_=hp[:, SD1 + H:SD1 + 2 * H])
                    nc.sync.dma_start(
                        out=d_all[:, (r * ST + st) * H:(r * ST + st + 1) * H],
                        in_=dpk[:])

            # ---------- own-shard d1 selection ----------
            dal = cpool.tile([P, cfg.CORES, ST, H], BF16)
            nc.sync.dma_start(
                out=dal[:],
                in_=d_all[:].rearrange("p (c s h) -> p c s h", c=cfg.CORES, s=ST))
            dmul = cpool.tile([P, cfg.CORES, ST, H], F32)
            nc.vector.tensor_tensor(
                out=dmul[:], in0=dal[:],
                in1=oh_bc[:].rearrange("p (c a b) -> p c a b", a=1, b=1).to_broadcast(
                    [P, cfg.CORES, ST, H]),
                op=mybir.AluOpType.mult)
            d1f = cpool.tile([P, ST, H], F32)
            nc.vector.tensor_reduce(
                out=d1f[:],
                in_=dmul[:].rearrange("p c s h -> p s h c"),
                axis=mybir.AxisListType.X, op=mybir.AluOpType.add)
            d1o = cpool.tile([P, ST, H], BF16)
            nc.vector.tensor_copy(out=d1o[:], in_=d1f[:])
            d2o = cpool.tile([P, ST, 1], BF16)
            g_all = cpool.tile([P, ST, H, cfg.HID], F32)
            fence1_t = cpool.tile([P, 1], F32)
            fence1 = nc.vector.memset(fence1_t[:], 0.0).ins
            for w in t1_writes:
                add_dep_helper(fence1, w, reason="table1 complete before gathers")
            fences = {1: fence1}
            _pp0cm.__exit__(None, None, None)
            _pp1cm = tc.tile_pool(name="psum_e", bufs=2, space="PSUM")
            ppool1 = _pp1cm.__enter__()
            _pp2cm = tc.tile_pool(name="psum_t", bufs=1, space="PSUM")
            ppool2 = _pp2cm.__enter__()

            # ---------- shared edge-pass ----------
            def edge_pass(layer):
                if layer == 1:
                    table, heads, scol = table1, H, SD1
                    mw = cfg.HID + 1   # per-head message width (h | ones)
                    down = d1o
                else:
                    table, heads, scol = t2_full, 1, cfg.OUT + 1
                    mw = cfg.OUT + 1
                    down = d2o
                for gi, g in enumerate(groups):
                    idx_t = gpool.tile([P, C16max], I16, tag="idx")
                    nc.sync.dma_start(out=idx_t[:, 0:g.c16], in_=idx_ext[gi, :, 0:g.c16])
                    m0_t = gpool.tile([P, CHmax * P], FP8, tag="m0")
                    nc.sync.dma_start(out=m0_t[:, 0:g.nch * P], in_=m0_ext[gi, :, 0:g.nch * P])
                    m0t_t = gpool.tile([P, CHmax * P], FP8, tag="m0t")
                    nc.sync.dma_start(out=m0t_t[:, 0:g.nch * P], in_=m0t_ext[gi, :, 0:g.nch * P])
                    gath = gpool.tile([P, CHmax, P], BF16, tag="gath")
                    for b in range(NB):
                        off16, nidx, ch0 = g.calls[b]
                        while nidx > 0:
                            n = min(nidx, 4096)
                            gi_inst = nc.gpsimd.dma_gather(
                                gath[:, ch0:ch0 + n // P, :],
                                table[b * BR:NT, :],
                                idx_t[:, off16:off16 + n // 16],
                                n, n, P, single_packet=False)
                            add_dep_helper(gi_inst.ins, fences[layer],
                                           reason="table ready before gather")
                            nidx -= n
                            ch0 += n // P
                            off16 += n // 16
                    tp = ppool1.tile([P, cfg.GST, Bmax, H], F32, space="PSUM", tag="tp_t")
                    aggp = ppool1.tile([P, cfg.GST, heads, mw], F32, space="PSUM", tag="tp_agg")
                    for (ci, sti, st_abs, c_st, first, last) in g.chunks:
                        nc.tensor.matmul(
                            out=tp[:, sti, c_st, 0:heads],
                            lhsT=m0t_t[:, ci * P:(ci + 1) * P],
                            rhs=down[:, st_abs, 0:heads],
                            start=True, stop=True)
                    ts_t = wpool.tile([P, cfg.GST, Bmax, H], F32, tag="ts")
                    ex_t = wpool.tile([P, cfg.GST, Bmax, H], F32, tag="ex")
                    for sti in range(len(g.sts)):
                        B_st = g.bst[sti]
                        if B_st == 0:
                            continue
                        for (b, ch0, c0, B) in g.struns[sti]:
                            if B == 0:
                                continue
                            nc.vector.tensor_tensor(
                                out=ts_t[:, sti, c0:c0 + B, 0:heads],
                                in0=tp[:, sti, c0:c0 + B, 0:heads],
                                in1=gath[:, ch0:ch0 + B, scol:scol + heads],
                                op=mybir.AluOpType.add)
                        nc.vector.tensor_scalar_mul(
                            out=ex_t[:, sti, 0:B_st, 0:heads],
                            in0=ts_t[:, sti, 0:B_st, 0:heads], scalar1=cfg.neg)
                        nc.vector.tensor_tensor(
                            out=ts_t[:, sti, 0:B_st, 0:heads],
                            in0=ts_t[:, sti, 0:B_st, 0:heads],
                            in1=ex_t[:, sti, 0:B_st, 0:heads],
                            op=mybir.AluOpType.max)
                        nc.scalar.activation(
                            out=ex_t[:, sti, 0:B_st, 0:heads],
                            in_=ts_t[:, sti, 0:B_st, 0:heads],
                            func=mybir.ActivationFunctionType.Exp)
                        for (b, ch0, c0, B) in g.struns[sti]:
                            if B == 0:
                                continue
                            for h in range(heads):
                                nc.vector.tensor_tensor(
                                    out=gath[:, ch0:ch0 + B, h * mw:(h + 1) * mw],
                                    in0=gath[:, ch0:ch0 + B, h * mw:(h + 1) * mw],
                                    in1=ex_t[:, sti, c0:c0 + B, h:h + 1].to_broadcast(
                                        [P, B, mw]),
                                    op=mybir.AluOpType.mult)
                    for (ci, sti, st_abs, c_st, first, last) in g.chunks:
                        nc.tensor.matmul(
                            out=aggp[:, sti, :, :].rearrange("p h m -> p (h m)"),
                            lhsT=m0_t[:, ci * P:(ci + 1) * P],
                            rhs=gath[:, ci, 0:heads * mw],
                            start=first, stop=last)
                    # normalize per supertile
                    for sti, st_abs in enumerate(g.sts):
                        rec = wpool.tile([P, heads, 1], F32, tag="rec")
                        # +eps: pad dst rows have zero denominators (no edges);
                        # keeps 0 * (1/eps) = 0 instead of 0 * inf = NaN
                        nc.vector.tensor_scalar_add(
                            out=rec[:], in0=aggp[:, sti, :, mw - 1:mw], scalar1=1e-30)
                        nc.vector.reciprocal(out=rec[:], in_=rec[:])
                        if layer == 1:
                            nc.vector.tensor_tensor(
                                out=g_all[:, st_abs, :, :],
                                in0=aggp[:, sti, :, 0:cfg.HID],
                                in1=rec[:].to_broadcast([P, heads, cfg.HID]),
                                op=mybir.AluOpType.mult)
                            nc.vector.tensor_tensor(
                                out=g_all[:, st_abs, :, :], in0=g_all[:, st_abs, :, :],
                                in1=b1_bc[:], op=mybir.AluOpType.add)
                            gv = g_all[:, st_abs, :, :].rearrange("p h d -> p (h d)")
                            if cfg.sim_gelu:
                                _gelu_tanh(nc, wpool, gv)
                            else:
                                nc.scalar.activation(
                                    out=gv, in_=gv,
                                    func=mybir.ActivationFunctionType.Gelu)
                        else:
                            ov = wpool.tile([P, cfg.OUT], F32, tag="ov")
                            nc.vector.tensor_tensor(
                                out=ov[:], in0=aggp[:, sti, 0, 0:cfg.OUT],
                                in1=rec[:, 0, :].to_broadcast([P, cfg.OUT]),
                                op=mybir.AluOpType.mult)
                            nc.vector.tensor_tensor(
                                out=ov[:], in0=ov[:], in1=b2_bc[:],
                                op=mybir.AluOpType.add)
                            rows = cfg.SHARD - st_abs * P if st_abs == ST - 1 else P
                            rows = min(P, rows)
                            nc.sync.dma_start(
                                out=out_ext[st_abs * P:st_abs * P + rows, :],
                                in_=ov[0:rows, :])

            edge_pass(1)

            # ---------- phase 1.5: table2 shard build ----------
            t2_writes = []
            for st in range(ST):
                rows = cfg.SHARD - st * P if st == ST - 1 else P
                rows = min(P, rows)
                gT_p = ppool2.tile([H * cfg.HID, P], F32, space="PSUM", tag="gT")
                nc.tensor.transpose(
                    out=gT_p[:], in_=g_all[:, st, :, :].rearrange("p h d -> p (h d)"),
                    identity=ident[:])
                gT = wpool.tile([H * cfg.HID, P], F32, tag="gTs")
                nc.scalar.activation(out=gT[:], in_=gT_p[:],
                                     func=mybir.ActivationFunctionType.Copy)
                h2p = ppool2.tile([P, NC2], F32, space="PSUM", tag="h2p")
                nc.tensor.matmul(out=h2p[:], lhsT=gT[:], rhs=w2e[:, :],
                                 start=True, stop=True)
                pack = wpool.tile([P, P], BF16, tag="pack")
                if cfg.sim_gelu:
                    nc.vector.memset(pack[:, cfg.OUT + 3:P], 0.0)
                nc.vector.tensor_copy(out=pack[:, 0:cfg.OUT], in_=h2p[:, 0:cfg.OUT])
                nc.vector.memset(pack[:, cfg.OUT:cfg.OUT + 1], 1.0)
                nc.vector.tensor_copy(out=pack[:, cfg.OUT + 1:cfg.OUT + 3],
                                      in_=h2p[:, cfg.OUT:NC2])
                nc.vector.tensor_copy(out=d2o[:, st, :], in_=h2p[:, NC2 - 1:NC2])
                t2_writes.append(nc.sync.dma_start(
                    out=t2_shard[st * P:st * P + P, :], in_=pack[:]).ins)

            cc_inst = nc.gpsimd.collective_compute(
                "AllGather", mybir.AluOpType.bypass,
                ins=[t2_shard[:].opt()], outs=[t2_full[:].opt()],
                replica_groups=[list(range(cfg.CORES))])
            for w in t2_writes:
                add_dep_helper(cc_inst.ins, w, reason="t2 shard complete before AG")
            fences[2] = cc_inst.ins

            edge_pass(2)
            _pp2cm.__exit__(None, None, None)
            _pp1cm.__exit__(None, None, None)

    nc.compile()
    return nc


def _gelu_tanh(nc, wpool, gv):
    """tanh-approx gelu in-place on gv [P, D] (CoreSim-compatible)."""
    Pp, D = gv.shape[0], gv.shape[1]
    t1 = wpool.tile([Pp, D], F32, tag="glu1")
    # 0.0356774*x^2 = (0.188885*x)^2
    nc.scalar.activation(out=t1[:], in_=gv, scale=0.1888856,
                         func=mybir.ActivationFunctionType.Square)
    nc.vector.tensor_scalar_add(out=t1[:], in0=t1[:], scalar1=0.7978846)
    nc.vector.tensor_tensor(out=t1[:], in0=t1[:], in1=gv, op=mybir.AluOpType.mult)
    nc.scalar.activation(out=t1[:], in_=t1[:],
                         func=mybir.ActivationFunctionType.Tanh)
    nc.vector.tensor_scalar_add(out=t1[:], in0=t1[:], scalar1=1.0)
    nc.vector.tensor_tensor(out=t1[:], in0=t1[:], in1=gv, op=mybir.AluOpType.mult)
    nc.vector.tensor_scalar_mul(out=gv, in0=t1[:], scalar1=0.5)


_CACHE = {}


def _get_built(cfg, edge_index):
    key = hash((edge_index.tobytes(), cfg.N, cfg.E, cfg.GST, cfg.sim_gelu))
    if key not in _CACHE:
        sched, arrays = preprocess(edge_index, cfg)
        nc = build_nc(cfg, sched)
        _CACHE[key] = (nc, sched, arrays)
    return _CACHE[key]


def make_in_maps(cfg, arrays, inputs):
    shared = dict(
        x=np.ascontiguousarray(inputs["x"], dtype=np.float32),
        W1=np.ascontiguousarray(inputs["W1"], dtype=np.float32),
        a_src1=np.ascontiguousarray(inputs["a_src1"], dtype=np.float32),
        a_dst1=np.ascontiguousarray(inputs["a_dst1"], dtype=np.float32),
        b1=np.ascontiguousarray(inputs["b1"], dtype=np.float32).reshape(1, -1),
        W2=np.ascontiguousarray(inputs["W2"], dtype=np.float32),
        a_src2=np.ascontiguousarray(inputs["a_src2"], dtype=np.float32),
        a_dst2=np.ascontiguousarray(inputs["a_dst2"], dtype=np.float32),
        b2=np.ascontiguousarray(inputs["b2"], dtype=np.float32).reshape(1, -1),
    )
    in_maps = []
    for r in range(cfg.CORES):
        m = dict(shared)
        m["idx"] = arrays[r]["idx"]
        m["m0"] = arrays[r]["m0"]
        m["m0t"] = arrays[r]["m0t"]
        m["onehot"] = arrays[r]["onehot"]
        in_maps.append(m)
    return in_maps


def kernel(x, edge_index, W1, a_src1, a_dst1, b1, W2, a_src2, a_dst2, b2,
           cfg=None, return_extras=False):
    from concourse.bass_utils import run_bass_kernel_spmd
    cfg = cfg or Cfg()
    nc, sched, arrays = _get_built(cfg, np.asarray(edge_index))
    in_maps = make_in_maps(cfg, arrays, dict(
        x=x, W1=W1, a_src1=a_src1, a_dst1=a_dst1, b1=b1,
        W2=W2, a_src2=a_src2, a_dst2=a_dst2, b2=b2))
    res = run_bass_kernel_spmd(nc, in_maps, list(range(cfg.CORES)))
    out = np.concatenate([res.results[r]["out"] for r in range(cfg.CORES)], axis=0)
    if return_extras:
        return out, res
    return out

